# revision 61
# baseline (speedup 1.0000x reference)
"""Trainium2 Bass kernel for a transformer decoder block (self-attn + cross-attn + MLP).

Sharding: 8 cores = 2 batch groups x 4 cores. Within a group, core c owns
rows r = c (mod 4) of its batch (strided rows balance causal attention work
while keeping the compiled program identical across cores). K/V are computed
replicated within a group (no collectives; cores are fully independent).

Layouts: activations bf16; weights host-cast to bf16 and pre-transposed to
[in_feat, out_feat] (q weights pre-scaled by SCALE); transposed activations
produced by the DMA xbar transpose into [128, tile, kin, 128] packing; logits
computed transposed [keys, rows] so exp(logits) feeds the A@V matmul directly;
the softmax denominator comes from an appended ones-column in V (row 64 of
the A@V output). All K/V activations stay resident in SBUF (no DRAM staging).
"""

import os
import sys

for _p in ("/opt/trn_rl_repo", "/root/.axon_site/_ro/trn_rl_repo"):
    if os.path.isdir(_p) and _p not in sys.path:
        sys.path.insert(0, _p)

import numpy as np

B, N, C, H, Y_DIM, HID = 2, 2048, 1024, 16, 1024, 4096
HD = C // H
SCALE = HD ** -0.5
EPS = 1e-5

G = 2          # batch groups
CPG = 4        # cores per group
R = N // CPG   # rows per core (512)
RT = R // 128  # row tiles per core (4)
KB = N // 128  # key blocks (16)
KIN = C // 128  # contraction tiles for C (8)
NT = N // 128  # full row tiles (16)
NEG = -1e9

_CACHE = {}


# ---------------------------------------------------------------------------
# program builder
# ---------------------------------------------------------------------------

def _build(mode, skip_gb, skip_bias):
    """mode: 'causal' | 'none' | 'dense'"""
    import concourse.bass as bass
    import concourse.mybir as mybir
    import concourse.tile as tile
    from concourse import bacc
    from concourse.masks import make_identity

    dt = mybir.dt
    F32, BF16 = dt.float32, dt.bfloat16
    AF = mybir.ActivationFunctionType
    ALU = mybir.AluOpType

    nc = bacc.Bacc("TRN2", target_bir_lowering=False, debug=False, num_devices=8)

    # ---- DRAM I/O ----------------------------------------------------------
    def din(name, shape, dtype=F32):
        return nc.dram_tensor(name, list(shape), dtype,
                              kind="ExternalInput").ap()

    x_my = din("x_my", (R, C))
    x_full = din("x_full", (N, C), BF16)
    yT = din("yT", (Y_DIM, N), BF16)
    wqkT = din("wqkT", (C, 2 * C), BF16)
    wvT = din("wvT", (C, C), BF16)
    wprojT = din("wprojT", (C, C), BF16)
    wq2T = din("wq2T", (C, C), BF16)
    wkv2T = din("wkv2T", (Y_DIM, 2 * C), BF16)
    wproj2T = din("wproj2T", (C, C), BF16)
    wfc1T = din("wfc1T", (C, HID), BF16)
    wfc2T = din("wfc2T", (HID, C), BF16)
    if not skip_bias:
        projb = din("projb", (C,))
        proj2b = din("proj2b", (C,))
        fc1b = din("fc1b", (HID,))
        fc2b = din("fc2b", (C,))
    if mode == "causal":
        bmask = din("bmask", (128, 32))
    if mode == "dense":
        maskT = din("maskT", (N, R))
    if not skip_gb:
        lng = {k: din("g_" + k, (HID if k == "mln2" else C,))
               for k in ("ln1", "aln2", "a2ln", "mln1", "mln2")}
        lnb = {k: din("b_" + k, (HID if k == "mln2" else C,))
               for k in ("ln1", "aln2", "a2ln", "mln1", "mln2")}
    out_my = nc.dram_tensor("out_my", [R, C], F32, kind="ExternalOutput").ap()

    def bcast(vec_ap, n):
        # DRAM [n] -> AP replicated across 128 partitions
        return bass.AP(tensor=vec_ap.tensor, offset=vec_ap.offset,
                       ap=[[0, 128]] + vec_ap.ap)

    def pkd(w_ap, nk, c0, w):
        # weight DRAM [(nk 128), cols] -> [128, nk, w] slab at col offset c0
        return w_ap.rearrange("(k p) c -> p k c", p=128)[:, :nk, c0:c0 + w]

    with tile.TileContext(nc) as tc:
        with tc.tile_pool(name="singles", bufs=1) as singles, \
             tc.tile_pool(name="stats", bufs=6) as stats, \
             tc.tile_pool(name="resid", bufs=1) as resid:

            ident = singles.tile([128, 128], F32, name="ident", tag="ident")
            make_identity(nc, ident)
            eps_t = singles.tile([128, 1], F32, name="eps", tag="eps")
            nc.vector.memset(eps_t, EPS)

            if mode == "causal":
                bmask_t = singles.tile([128, 32], F32, name="bmask", tag="bmask")
                nc.sync.dma_start(out=bmask_t, in_=bmask)
                # multiplicative 0/1 boundary mask (bf16), applied post-exp
                em01b = singles.tile([128, 32], BF16, name="em01b", tag="em01b")
                nc.vector.tensor_copy(out=em01b, in_=bmask_t)
            identb = singles.tile([128, 128], BF16, name="identb", tag="identb")
            make_identity(nc, identb)
            maskT_t = None
            if mode == "dense":
                maskT_t = [singles.tile([128, R], F32, name=f"maskT{j}", tag=f"maskT{j}")
                           for j in range(KB)]
                for j in range(KB):
                    nc.sync.dma_start(out=maskT_t[j], in_=maskT[j * 128:(j + 1) * 128, :])

            gb_tiles = {}
            if not skip_gb:
                for k in ("ln1", "aln2", "a2ln", "mln1", "mln2"):
                    d = HID if k == "mln2" else C
                    gt = singles.tile([128, d], F32, name=f"g_{k}", tag=f"g_{k}")
                    bt = singles.tile([128, d], F32, name=f"b_{k}", tag=f"b_{k}")
                    nc.sync.dma_start(out=gt, in_=bcast(lng[k], d))
                    nc.sync.dma_start(out=bt, in_=bcast(lnb[k], d))
                    gb_tiles[k] = (gt, bt)
            bias_tiles = {}
            if not skip_bias:
                for nm_, ap_, d in (("projb", projb, C), ("proj2b", proj2b, C),
                                    ("fc1b", fc1b, HID), ("fc2b", fc2b, C)):
                    bt = singles.tile([128, d], F32, name=nm_, tag=nm_)
                    nc.sync.dma_start(out=bt, in_=bcast(ap_, d))
                    bias_tiles[nm_] = bt

            # ---- helpers ---------------------------------------------------
            def ln_group(xs, d):
                """Batched LN stats for a group of [128, d] tiles: all DVE
                stats first, then ONE sqrt / reciprocal for the whole group
                (avoids per-tile DVE<->ACT ping-pong serialization).
                Returns (ri, nm): [128, len(xs)] scale / bias columns."""
                n = len(xs)
                nsub = max(1, d // 512)
                mvall = stats.tile([128, n, 2], F32, name="mvall", tag="mvall")
                for i, x in enumerate(xs):
                    st = stats.tile([128, nsub, 6], F32, name="bnst", tag="bnst")
                    if nsub > 1:
                        xr = x.rearrange("p (s q) -> p s q", s=nsub)
                        for s in range(nsub):
                            nc.vector.bn_stats(out=st[:, s, :], in_=xr[:, s, :])
                    else:
                        nc.vector.bn_stats(out=st[:, 0, :], in_=x)
                    nc.vector.bn_aggr(out=mvall[:, i, :], in_=st)
                sd = stats.tile([128, n], F32, name="sdall", tag="sdall")
                nc.scalar.activation(out=sd, in_=mvall[:, :, 1], func=AF.Sqrt,
                                     bias=eps_t)
                ri = stats.tile([128, n], F32, name="riall", tag="riall")
                nc.vector.reciprocal(out=ri, in_=sd)
                nm = stats.tile([128, n], F32, name="nmall", tag="nmall")
                nc.vector.tensor_tensor(out=nm, in0=mvall[:, :, 0], in1=ri,
                                        op=ALU.mult)
                nc.vector.tensor_scalar(out=nm, in0=nm, scalar1=-1.0,
                                        scalar2=None, op0=ALU.mult)
                return ri, nm

            def ln_apply_i(h_out, x_in, d, key, ri, nm, i):
                nc.scalar.activation(out=h_out, in_=x_in, func=AF.Identity,
                                     bias=nm[:, i:i + 1], scale=ri[:, i:i + 1])
                if not skip_gb:
                    gt, bt = gb_tiles[key]
                    nc.vector.tensor_tensor(out=h_out, in0=h_out, in1=gt[:, :d],
                                            op=ALU.mult)
                    nc.vector.tensor_tensor(out=h_out, in0=h_out, in1=bt[:, :d],
                                            op=ALU.add)

            def ln_apply(h_out, x_in, d, key):
                ri, nm = ln_group([x_in], d)
                ln_apply_i(h_out, x_in, d, key, ri, nm, 0)

            def load_wslab(wtp, w_ap, col0, mh, name="w"):
                wts = wtp.tile([128, KIN, 512], BF16, name=name, tag="wslab")
                nc.gpsimd.dma_start(out=wts, in_=pkd(w_ap, KIN, col0 + mh * 512, 512))
                return wts

            def build_qT(dst, wtp, psp, w_ap, col0, srcT, pre=None, tgran=False):
                """dst: 8 tiles [128, R] bf16 = (W[:, col0:col0+C]).T @ act.T.
                srcT [128, RT, KIN, 128] bf16. tgran: per-row-tile matmuls so
                work starts as soon as srcT[:, 0] exists (one psum zero
                region: only the very first matmul carries start=True)."""
                for mh in range(2):
                    wts = pre[mh] if pre else load_wslab(wtp, w_ap, col0, mh, "qw")
                    for mm in range(4):
                        ps = psp.tile([128, R], F32, name="qps", tag="bps")
                        if tgran:
                            for t in range(RT):
                                for K in range(KIN):
                                    nc.tensor.matmul(
                                        ps[:, t * 128:(t + 1) * 128],
                                        wts[:, K, mm * 128:(mm + 1) * 128],
                                        srcT[:, t, K, :],
                                        start=(t == 0 and K == 0),
                                        stop=(t == RT - 1 and K == KIN - 1),
                                        skip_group_check=True)
                        else:
                            for K in range(KIN):
                                nc.tensor.matmul(
                                    ps, wts[:, K, mm * 128:(mm + 1) * 128],
                                    srcT[:, :, K, :],
                                    start=(K == 0), stop=(K == KIN - 1))
                        nc.vector.tensor_copy(out=dst[mh * 4 + mm], in_=ps)

            def build_kT(dst, wtp, psp, w_ap, col0, srcT, ncp=4):
                """dst: 8 tiles [128, N] bf16. srcT [128, NT, KIN, 128] bf16.
                ncp: column tiles (of 128 keys) per psum chunk."""
                cw = 128 * ncp
                for mh in range(2):
                    wts = load_wslab(wtp, w_ap, col0, mh, "kw")
                    for n in range(N // cw):
                        for mm in range(4):
                            ps = psp.tile([128, cw], F32, name="kps", tag="bps")
                            for K in range(KIN):
                                nc.tensor.matmul(
                                    ps, wts[:, K, mm * 128:(mm + 1) * 128],
                                    srcT[:, ncp * n:ncp * (n + 1), K, :],
                                    start=(K == 0), stop=(K == KIN - 1))
                            nc.vector.tensor_copy(
                                out=dst[mh * 4 + mm][:, n * cw:(n + 1) * cw], in_=ps)

            def build_v(dst, wtp, psp, w_ap, col0, srcT):
                """dst: NT tiles [128, H*65] bf16 (row-major V, ones col at 64).
                srcT [128, NT, KIN, 128] bf16."""
                for half in range(2):
                    wts = load_wslab(wtp, w_ap, col0, half, "vw")
                    for t in range(NT):
                        ps = psp.tile([128, 512], F32, name="vps", tag="bps")
                        for K in range(KIN):
                            nc.tensor.matmul(ps, srcT[:, t, K, :], wts[:, K, :],
                                             start=(K == 0), stop=(K == KIN - 1))
                        dr = dst[t].rearrange("p (h c) -> p h c", c=65)
                        nc.vector.tensor_copy(
                            out=dr[:, half * 8:(half + 1) * 8, 0:64],
                            in_=ps.rearrange("p (h c) -> p h c", c=64))
                for t in range(NT):
                    dr = dst[t].rearrange("p (h c) -> p h c", c=65)
                    nc.gpsimd.memset(dr[:, :, 64:65], 1.0)

            def rows_matmul(wtp, psp, lhsT, w_ap, dout, nkt, consume):
                """out[rows, dout] = act @ W.T. lhsT [128, RT, nkt, 128] bf16.
                consume(rt, nch, psum [128, 512]). rt-outer so the first psum
                group only needs lhsT[:, 0] (not all row tiles)."""
                nslab = nkt // 8
                for nch in range(dout // 512):
                    wts = []
                    for Kg in range(nslab):
                        wt = wtp.tile([128, 8, 512], BF16, name="dw",
                                      tag=f"dw{nslab}", bufs=nslab + 1)
                        nc.gpsimd.dma_start(
                            out=wt,
                            in_=w_ap.rearrange("(k p) c -> p k c", p=128)[
                                :, Kg * 8:(Kg + 1) * 8,
                                nch * 512:(nch + 1) * 512])
                        wts.append(wt)
                    for rt in range(RT):
                        ps = psp.tile([128, 512], F32, name="dps", tag="dps",
                                      bufs=2)
                        for K in range(nkt):
                            nc.tensor.matmul(
                                ps, lhsT[:, rt, K, :], wts[K // 8][:, K % 8, :],
                                start=(K == 0), stop=(K == nkt - 1))
                        consume(rt, nch, ps)

            def attention(qT, kT, v_t, o_sb, causal, masked=False, filler=None,
                          tail_hook=None):
                """o_sb: RT tiles [128, C] bf16 <- softmax(qk + mask) @ v."""
                lg_bufs = 3
                with tc.tile_pool(name="attp", bufs=lg_bufs, space="PSUM") as lgp, \
                     tc.tile_pool(name="attops", bufs=1, space="PSUM") as opsp, \
                     tc.tile_pool(name="attw", bufs=4) as aw:
                    o_ps_prev = [None]

                    def emit_tail(hp, o_ps, tile_hook=None):
                        ots = []
                        for hh in range(2):
                            ot = aw.tile([65, R], F32, name="otsb", tag="otsb",
                                         bufs=3)
                            nc.vector.tensor_copy(out=ot, in_=o_ps[hh])
                            ots.append(ot)
                        # t-outer so tile_hook(t) can fire as soon as o_sb[t]
                        # is fully written (last head-pair only)
                        for t in range(RT):
                            for hh in range(2):
                                h = 2 * hp + hh
                                tp = lgp.tile([128, 65], F32, name="otp",
                                              tag="logits")
                                nc.tensor.transpose(
                                    tp, ots[hh][:, t * 128:(t + 1) * 128],
                                    ident[0:65, 0:65])
                                ri = stats.tile([128, 1], F32, name="osum",
                                                tag="osum")
                                nc.vector.reciprocal(out=ri, in_=tp[:, 64:65])
                                nc.vector.tensor_scalar(
                                    out=o_sb[t][:, h * 64:(h + 1) * 64],
                                    in0=tp[:, 0:64], scalar1=ri,
                                    scalar2=None, op0=ALU.mult)
                            if tile_hook is not None:
                                tile_hook(t)

                    for hp in range(H // 2):
                        o_ps = [opsp.tile([65, R], F32, name=f"ops{hh}",
                                          tag=f"ops{hh}") for hh in range(2)]
                        lg_t = [None] * KB
                        pt_t = [None] * KB

                        def emit_lg(J):
                            r0 = 32 * J if causal else 0
                            nj = R - r0
                            lg = lgp.tile([128, 2, 512], F32, name="logits",
                                          tag="logits")
                            for hh in range(2):
                                nc.tensor.matmul(
                                    lg[:, hh, 0:nj],
                                    kT[hp][hh * 64:hh * 64 + 64, J * 128:(J + 1) * 128],
                                    qT[hp][hh * 64:hh * 64 + 64, r0:R])
                            if masked:
                                mt = maskT_t[J]
                                mk = bass.AP(tensor=mt.tensor, offset=mt.offset,
                                             ap=[mt.ap[0], [0, 2], mt.ap[1]])
                                nc.vector.tensor_tensor(out=lg[:, :, 0:nj],
                                                        in0=lg[:, :, 0:nj],
                                                        in1=mk, op=ALU.add)
                            lg_t[J] = lg

                        def emit_exp(J):
                            r0 = 32 * J if causal else 0
                            nj = R - r0
                            pt = aw.tile([128, 2, 512], BF16, name="probs",
                                         tag="probs")
                            nc.scalar.activation(out=pt[:, :, 0:nj],
                                                 in_=lg_t[J][:, :, 0:nj],
                                                 func=AF.Exp)
                            if causal:
                                # zero the masked boundary probs (off the
                                # lg->exp->av critical chain)
                                bm = bass.AP(tensor=em01b.tensor,
                                             offset=em01b.offset,
                                             ap=[em01b.ap[0], [0, 2], em01b.ap[1]])
                                nc.vector.tensor_tensor(out=pt[:, :, 0:32],
                                                        in0=pt[:, :, 0:32],
                                                        in1=bm, op=ALU.mult)
                            lg_t[J] = None
                            pt_t[J] = pt

                        def emit_av_main(J):
                            # bulk columns [r0+32:R] (fully unmasked probs)
                            r0 = 32 * J if causal else 0
                            nj = R - r0
                            c0 = 32 if causal else 0
                            if nj <= c0:
                                return
                            for hh in range(2):
                                h = 2 * hp + hh
                                nc.tensor.matmul(o_ps[hh][:, r0 + c0:R],
                                                 v_t[J][:, h * 65:h * 65 + 65],
                                                 pt_t[J][:, hh, c0:nj],
                                                 start=(J == 0),
                                                 stop=(J == KB - 1 and not causal),
                                                 skip_group_check=causal)
                            if not causal:
                                pt_t[J] = None

                        def emit_av_bnd(J):
                            # boundary columns [r0:r0+32] (masked probs).
                            # start stays False: av_main(0)'s start already
                            # marked the whole 2KB zero region pending-zero,
                            # so these bytes overwrite on first touch; a second
                            # start=True would re-mark (and discard) av_main's
                            # accumulated columns.
                            r0 = 32 * J
                            for hh in range(2):
                                h = 2 * hp + hh
                                nc.tensor.matmul(o_ps[hh][:, r0:r0 + 32],
                                                 v_t[J][:, h * 65:h * 65 + 65],
                                                 pt_t[J][:, hh, 0:32],
                                                 start=False,
                                                 stop=(J == KB - 1),
                                                 skip_group_check=True)
                            pt_t[J] = None

                        look = lg_bufs - 1
                        for j0 in range(min(look, KB)):
                            emit_lg(j0)
                        if o_ps_prev[0] is not None:
                            emit_tail(hp - 1, o_ps_prev[0])
                        for J in range(KB):
                            if J + look < KB:
                                emit_lg(J + look)
                            emit_exp(J)
                            emit_av_main(J)
                            if causal and J >= 1:
                                emit_av_bnd(J - 1)
                            if filler is not None and J % 4 == 3 and J < KB - 1:
                                filler(hp, J // 4, lgp)
                        if causal:
                            emit_av_bnd(KB - 1)
                        if filler is not None:
                            filler(hp, KB // 4 - 1, lgp)
                        o_ps_prev[0] = o_ps
                    if filler is not None:
                        for sub in range(4):
                            filler(H // 2, sub, lgp)
                    emit_tail(H // 2 - 1, o_ps_prev[0], tile_hook=tail_hook)

            # ================================================================
            # residual-stream tiles
            # ================================================================
            x_my_t = [resid.tile([128, C], F32, name=f"xmy{t}", tag=f"xmy{t}")
                      for t in range(RT)]
            x1_my = [resid.tile([128, C], F32, name=f"x1my{t}", tag=f"x1my{t}")
                     for t in range(RT)]
            # x2 reuses x_my's slots (x_my dead once the self-attn proj consumed it)
            x2_my = [resid.tile([128, C], F32, name=f"x2my{t}", tag=f"xmy{t}")
                     for t in range(RT)]

            # ================================================================
            # Era 1: attention (self + cross)
            # ================================================================
            with tc.tile_pool(name="era1", bufs=1) as e1:
                hmyT = e1.tile([128, RT, KIN, 128], BF16, name="hmyT", tag="tr8")
                qT = [e1.tile([128, R], BF16, name=f"qT{m}", tag=f"qT{m}")
                      for m in range(KIN)]
                kT = [e1.tile([128, N], BF16, name=f"kT{m}", tag=f"kT{m}")
                      for m in range(KIN)]
                v_t = [e1.tile([128, H * 65], BF16, name=f"v{t}", tag=f"v{t}")
                       for t in range(NT)]
                o_sb = [e1.tile([128, C], BF16, name=f"osb{t}", tag=f"osb{t}")
                        for t in range(RT)]
                hT = e1.tile([128, NT, KIN, 128], BF16, name="hT", tag="big")

                # ---- h/hmy: LN1 + transpose ------------------------------
                with tc.tile_pool(name="s0w", bufs=3) as s0w, \
                     tc.tile_pool(name="aw1", bufs=2) as wtp, \
                     tc.tile_pool(name="aps1", bufs=3, space="PSUM") as psp:
                    # x_my rows first, then q weight slabs, then the bulk
                    # x_full tiles (DMA engines drain in arrival order; q's
                    # dependencies must land early so PE can start)
                    for t in range(RT):
                        nc.gpsimd.dma_start(out=x_my_t[t],
                                            in_=x_my[t * 128:(t + 1) * 128, :])
                    qw_pre = [load_wslab(wtp, wqkT, 0, mh, "qw") for mh in range(2)]
                    xfs = []
                    for t in range(NT):
                        xf = s0w.tile([128, C], BF16, name="xfull", tag="xfull",
                                      bufs=6)
                        nc.gpsimd.dma_start(out=xf,
                                            in_=x_full[t * 128:(t + 1) * 128, :])
                        xfs.append(xf)
                    # hmyT via PE transposes (PE is idle; keeps the xbar free)
                    ri_m, nm_m = ln_group(x_my_t, C)
                    for t in range(RT):
                        hm = s0w.tile([128, C], BF16, name="hmy", tag="hmy",
                                      bufs=2)
                        ln_apply_i(hm, x_my_t[t], C, "ln1", ri_m, nm_m, t)
                        ptb = psp.tile([128, KIN, 128], BF16, name="trps",
                                       tag="trps", bufs=2)
                        for k in range(KIN):
                            nc.tensor.transpose(
                                ptb[:, k, :], hm[:, k * 128:(k + 1) * 128], identb)
                        nc.vector.tensor_copy(out=hmyT[:, t], in_=ptb)
                    for g in range(NT // 4):
                        ri_f, nm_f = ln_group(xfs[4 * g:4 * g + 4], C)
                        for j in range(4):
                            t = 4 * g + j
                            hf = s0w.tile([128, C], BF16, name="hfull",
                                          tag="hfull")
                            ln_apply_i(hf, xfs[t], C, "ln1", ri_f, nm_f, j)
                            nc.sync.dma_start(out=hT[:, t], in_=hf,
                                              transpose=True)

                    # ---- q/k/v builds ------------------------------------
                    build_qT(qT, wtp, psp, wqkT, 0, hmyT, pre=qw_pre)
                    build_kT(kT, wtp, psp, wqkT, C, hT)
                    build_v(v_t, wtp, psp, wvT, 0, hT)

                # yT into SBUF, reusing hT's slot (WAR: waits for v/k builds)
                yT_sb = e1.tile([128, KIN, N], BF16, name="yTsb", tag="big")
                nc.gpsimd.dma_start(out=yT_sb,
                                    in_=yT.rearrange("(k p) n -> p k n", p=128))
                yTr = yT_sb.rearrange("p k (t n) -> p t k n", n=128)

                # cross-attn K build interleaved into self-attention: k2T[m]
                # reuses kT[m]'s slot, which frees exactly when head-pair m
                # finishes its logits — the filler for hp emits unit m=hp.
                k2T = [None] * KIN
                fill_state = {}
                with tc.tile_pool(name="fillw", bufs=2) as fwtp:

                    def k2_filler(hp, sub, lgp):
                        # build unit m = hp-1: kT[m]'s slot frees only once
                        # head-pair m has read all its logits, so the unit for
                        # m must trail its own hp by one iteration
                        m = hp - 1
                        if m < 0:
                            return
                        if sub == 0 and m % 4 == 0:
                            ws = fwtp.tile([128, KIN, 512], BF16, name="k2w",
                                           tag="fwslab")
                            nc.gpsimd.dma_start(
                                out=ws, in_=pkd(wkv2T, KIN, (m // 4) * 512, 512))
                            fill_state["slab"] = ws
                        if sub == 0:
                            k2T[m] = e1.tile([128, N], BF16, name=f"k2T{m}",
                                             tag=f"kT{m}")
                        wts = fill_state["slab"]
                        n = sub
                        # borrow a slot from the logits ring (PSUM is full)
                        ps = lgp.tile([128, 512], F32, name="k2ps", tag="logits")
                        for K in range(KIN):
                            nc.tensor.matmul(
                                ps, wts[:, K, (m % 4) * 128:(m % 4 + 1) * 128],
                                yTr[:, 4 * n:4 * (n + 1), K, :],
                                start=(K == 0), stop=(K == KIN - 1))
                        nc.vector.tensor_copy(
                            out=k2T[m][:, n * 512:(n + 1) * 512], in_=ps)

                    attention(qT, kT, v_t, o_sb, causal=(mode == "causal"),
                              masked=(mode == "dense"), filler=k2_filler)

                # ---- cross k/v builds (independent of self-attn output),
                # then self proj + q2, whose LN/DVE chains hide under them.
                with tc.tile_pool(name="x1w", bufs=2) as wtp, \
                     tc.tile_pool(name="x1work", bufs=2) as pw, \
                     tc.tile_pool(name="x1ps", bufs=3, space="PSUM") as psp, \
                     tc.tile_pool(name="p1ps", bufs=1, space="PSUM") as psp1:
                    v2_t = [e1.tile([128, H * 65], BF16, name=f"v2{t}", tag=f"v{t}")
                            for t in range(NT)]
                    build_v(v2_t, wtp, psp, wkv2T, C, yTr)

                    olnT = e1.tile([128, RT, KIN, 128], BF16, name="olnT",
                                   tag="tr8")
                    ri_o, nm_o = ln_group(o_sb, C)
                    for t in range(RT):
                        oln = pw.tile([128, C], BF16, name="oln", tag="oln")
                        ln_apply_i(oln, o_sb[t], C, "aln2", ri_o, nm_o, t)
                        nc.sync.dma_start(out=olnT[:, t], in_=oln, transpose=True)

                    def consume_proj(rt, nch, ps):
                        sl = slice(nch * 512, (nch + 1) * 512)
                        if skip_bias:
                            nc.vector.tensor_tensor(out=x1_my[rt][:, sl], in0=ps,
                                                    in1=x_my_t[rt][:, sl],
                                                    op=ALU.add)
                        else:
                            nc.vector.tensor_tensor(out=x1_my[rt][:, sl], in0=ps,
                                                    in1=bias_tiles["projb"][:, sl],
                                                    op=ALU.add)
                            nc.vector.tensor_tensor(out=x1_my[rt][:, sl],
                                                    in0=x1_my[rt][:, sl],
                                                    in1=x_my_t[rt][:, sl],
                                                    op=ALU.add)

                    rows_matmul(wtp, psp1, olnT, wprojT, C, KIN, consume_proj)

                    h2T = e1.tile([128, RT, KIN, 128], BF16, name="h2T",
                                  tag="tr8")
                    for t in range(RT):
                        h2 = pw.tile([128, C], BF16, name="h2", tag="h2")
                        ln_apply(h2, x1_my[t], C, "a2ln")
                        nc.sync.dma_start(out=h2T[:, t], in_=h2, transpose=True)
                    q2T = [e1.tile([128, R], BF16, name=f"q2T{m}", tag=f"qT{m}")
                           for m in range(KIN)]
                    build_qT(q2T, wtp, psp, wq2T, 0, h2T, tgran=True)

                o2_sb = [e1.tile([128, C], BF16, name=f"o2sb{t}", tag=f"osb{t}")
                         for t in range(RT)]
                o2T = e1.tile([128, RT, KIN, 128], BF16, name="o2T", tag="tr8")

                def o2_tail_hook(t):
                    # o2_sb[t] is complete the moment the last head-pair's
                    # tail writes it; start its transpose immediately so
                    # proj2 can begin before the whole tail drains
                    nc.sync.dma_start(out=o2T[:, t], in_=o2_sb[t],
                                      transpose=True)

                attention(q2T, k2T, v2_t, o2_sb, causal=False,
                          tail_hook=o2_tail_hook)

                h3T = resid.tile([128, RT, KIN, 128], BF16, name="h3T",
                                 tag="h3T")
                with tc.tile_pool(name="p2w", bufs=2) as wtp, \
                     tc.tile_pool(name="p2work", bufs=2) as p2w, \
                     tc.tile_pool(name="p2ps", bufs=2, space="PSUM") as psp:

                    def consume_proj2(rt, nch, ps):
                        sl = slice(nch * 512, (nch + 1) * 512)
                        if skip_bias:
                            nc.vector.tensor_tensor(out=x2_my[rt][:, sl], in0=ps,
                                                    in1=x1_my[rt][:, sl],
                                                    op=ALU.add)
                        else:
                            nc.vector.tensor_tensor(out=x2_my[rt][:, sl], in0=ps,
                                                    in1=bias_tiles["proj2b"][:, sl],
                                                    op=ALU.add)
                            nc.vector.tensor_tensor(out=x2_my[rt][:, sl],
                                                    in0=x2_my[rt][:, sl],
                                                    in1=x1_my[rt][:, sl],
                                                    op=ALU.add)
                        if nch == C // 512 - 1:
                            # x2[rt] complete: overlap its MLP LN + transpose
                            # with the remaining proj2 matmuls
                            h3 = p2w.tile([128, C], BF16, name="h3", tag="h3")
                            ln_apply(h3, x2_my[rt], C, "mln1")
                            nc.sync.dma_start(out=h3T[:, rt], in_=h3,
                                              transpose=True)

                    rows_matmul(wtp, psp, o2T, wproj2T, C, KIN, consume_proj2)

            # ================================================================
            # Era 2: MLP
            # ================================================================
            with tc.tile_pool(name="era2", bufs=1) as e2, \
                 tc.tile_pool(name="mw", bufs=2) as wtp, \
                 tc.tile_pool(name="mwork", bufs=2) as mw, \
                 tc.tile_pool(name="mps", bufs=2, space="PSUM") as psp:
                h4 = e2.tile([128, RT, HID], BF16, name="h4", tag="h4")

                def consume_fc1(rt, nch, ps):
                    sl = slice(nch * 512, (nch + 1) * 512)
                    if not skip_bias:
                        nc.vector.tensor_tensor(
                            out=ps, in0=ps, in1=bias_tiles["fc1b"][:, sl],
                            op=ALU.add)
                    nc.scalar.activation(out=h4[:, rt, sl], in_=ps, func=AF.Gelu)

                rows_matmul(wtp, psp, h3T, wfc1T, HID, KIN, consume_fc1)

                h5T = e2.tile([128, RT, HID // 128, 128], BF16, name="h5T",
                              tag="h5T")
                # per-tile LN here (not grouped): fc2's first row tile only
                # needs h5T[:, 0], so don't gate it on all four tiles' stats
                for t in range(RT):
                    h5 = mw.tile([128, HID], BF16, name="h5", tag="h5")
                    ln_apply(h5, h4[:, t, :], HID, "mln2")
                    nc.sync.dma_start(out=h5T[:, t], in_=h5, transpose=True)

                def consume_fc2(rt, nch, ps):
                    sl = slice(nch * 512, (nch + 1) * 512)
                    x3 = mw.tile([128, 512], F32, name="x3", tag="x3")
                    if skip_bias:
                        nc.vector.tensor_tensor(out=x3, in0=ps,
                                                in1=x2_my[rt][:, sl], op=ALU.add)
                    else:
                        nc.vector.tensor_tensor(out=x3, in0=ps,
                                                in1=bias_tiles["fc2b"][:, sl],
                                                op=ALU.add)
                        nc.vector.tensor_tensor(out=x3, in0=x3,
                                                in1=x2_my[rt][:, sl], op=ALU.add)
                    nc.sync.dma_start(out=out_my[rt * 128:(rt + 1) * 128, sl],
                                      in_=x3)

                rows_matmul(wtp, psp, h5T, wfc2T, C, HID // 128, consume_fc2)

    nc.compile()
    return nc


# ---------------------------------------------------------------------------
# host side
# ---------------------------------------------------------------------------

def _bf16(a):
    import ml_dtypes
    return np.asarray(a, np.float32).astype(ml_dtypes.bfloat16)


def _host_prep(inputs):
    f32 = np.float32
    x = np.asarray(inputs["x"], f32)
    y = np.asarray(inputs["y"], f32)
    mask = np.asarray(inputs["mask"])[0, 0]  # [N, N] bool

    causal_ref = np.triu(np.ones((N, N), bool), k=1)
    if np.array_equal(mask, causal_ref):
        mode = "causal"
    elif not mask.any():
        mode = "none"
    else:
        mode = "dense"

    gbs = [("a1_ln1_g", "a1_ln1_b"), ("a1_ln2_g", "a1_ln2_b"),
           ("a2_ln_g", "a2_ln_b"), ("m_ln1_g", "m_ln1_b"), ("m_ln2_g", "m_ln2_b")]
    skip_gb = all(
        np.all(np.asarray(inputs[g]) == 1.0) and np.all(np.asarray(inputs[b]) == 0.0)
        for g, b in gbs)
    skip_bias = all(
        np.all(np.asarray(inputs[b]) == 0.0)
        for b in ("a1_proj_b", "a2_proj_b", "m_fc1_b", "m_fc2_b"))

    wT = lambda k: np.ascontiguousarray(np.asarray(inputs[k], f32).T)
    wqkT = wT("a1_qk_w")
    wqkT[:, :C] *= SCALE          # fold the attention scale into the q weights
    wq2T = wT("a2_q_w") * SCALE
    shared = {
        "wqkT": _bf16(wqkT),      # [C, 2C]: cols 0:C = q (pre-scaled), C:2C = k
        "wvT": _bf16(wT("a1_v_w")),
        "wprojT": _bf16(wT("a1_proj_w")),
        "wq2T": _bf16(wq2T),
        "wkv2T": _bf16(wT("a2_kv_w")),   # [Y, 2C]: cols 0:C = k, C:2C = v
        "wproj2T": _bf16(wT("a2_proj_w")),
        "wfc1T": _bf16(wT("m_fc1_w")),
        "wfc2T": _bf16(wT("m_fc2_w")),
    }
    if not skip_bias:
        shared["projb"] = np.asarray(inputs["a1_proj_b"], f32)
        shared["proj2b"] = np.asarray(inputs["a2_proj_b"], f32)
        shared["fc1b"] = np.asarray(inputs["m_fc1_b"], f32)
        shared["fc2b"] = np.asarray(inputs["m_fc2_b"], f32)
    if not skip_gb:
        keymap = {"ln1": ("a1_ln1_g", "a1_ln1_b"), "aln2": ("a1_ln2_g", "a1_ln2_b"),
                  "a2ln": ("a2_ln_g", "a2_ln_b"), "mln1": ("m_ln1_g", "m_ln1_b"),
                  "mln2": ("m_ln2_g", "m_ln2_b")}
        for k, (gk, bk) in keymap.items():
            shared["g_" + k] = np.asarray(inputs[gk], f32)
            shared["b_" + k] = np.asarray(inputs[bk], f32)

    in_maps = []
    for core in range(G * CPG):
        g, c = core // CPG, core % CPG
        m = dict(shared)
        m["x_my"] = np.ascontiguousarray(x[g, c::CPG])
        m["x_full"] = _bf16(x[g])
        m["yT"] = _bf16(np.ascontiguousarray(y[g].T))
        if mode == "causal":
            # boundary block: key kk (0..127) vs local row ii (0..31):
            # allowed iff kk <= c + 4*ii (multiplicative 0/1, applied post-exp)
            kk = np.arange(128)[:, None]
            ii = np.arange(32)[None, :]
            m["bmask"] = (kk <= c + CPG * ii).astype(f32)
        if mode == "dense":
            sub = mask[c::CPG, :]  # [R, N] rows of this core vs all keys
            m["maskT"] = np.ascontiguousarray(np.where(sub, NEG, 0.0).astype(f32).T)
        in_maps.append(m)
    return mode, skip_gb, skip_bias, in_maps


def _assemble(results, dtype):
    out = np.empty((B, N, C), np.float32)
    for core in range(G * CPG):
        g, c = core // CPG, core % CPG
        out[g, c::CPG] = results[core]["out_my"]
    return out.astype(dtype, copy=False)


def get_program(inputs):
    """Build (or fetch cached) program + per-core input maps for these inputs."""
    mode, skip_gb, skip_bias, in_maps = _host_prep(inputs)
    key = (mode, skip_gb, skip_bias)
    if key not in _CACHE:
        _CACHE[key] = _build(mode, skip_gb, skip_bias)
    return _CACHE[key], in_maps


def kernel(**inputs):
    from concourse import bass_utils

    nc, in_maps = get_program(inputs)
    res = bass_utils.run_bass_kernel_spmd(nc, in_maps, core_ids=list(range(8)))
    return _assemble(res.results, np.asarray(inputs["x"]).dtype)
